# revision 18
# baseline (speedup 1.0000x reference)
"""Trainium2 Bass kernel for ContinuousFilterConv (SchNet cfconv-style).

Computes, for each frame b and atom a:
    filt  = tanh(rbf[b,a,:,:] @ W1 + b1) @ W2 + b2          # [N, F]
    out[b,a,:] = sum_n filt[n,:] * features[b, nl[b,a,n], :]

Sharding: data-parallel over the 32 frames -> 8 NeuronCores x 4 frames.

Per-core pipeline:
  - rbf is pre-cast to bf16 and pre-transposed on the host into the
    matmul operand layout (gaussian dim on partitions, row-pair packed),
    so it streams in via plain HWDGE DMA with no on-device transpose.
  - mm1 (K=64, two PE-quadrant matmuls) -> tanh+b1 on the scalar engine
    (bf16 out) -> mm2 vs W2 in bf16.
  - neighbor gather is split: NSW slabs/frame use the SWDGE dma_gather
    (gpsimd desc-gen bound, ~33us/slab); the rest use a one-hot matmul
    on the PE (host uploads a one-hot encoding of neighbor_list; the PE
    computes feat^T @ onehot in 4 K=128 passes), staged PSUM->SBUF bf16
    by the scalar engine.
  - one fused DVE op computes (mm2_psum + b2) * gathered, then a 6-level
    pairwise tensor_tensor tree (2x bf16 mode) reduces 64 neighbors per
    atom (tensor_reduce is 1x-only on DVE, the tree is faster).
  - output is written untransposed [F, A]; the host transposes.
"""
import sys

for _p in ("/opt/trn_rl_repo", "/root/.axon_site/_ro/trn_rl_repo"):
    if _p not in sys.path:
        sys.path.insert(0, _p)

import numpy as np
import ml_dtypes

import concourse.bacc as bacc
import concourse.mybir as mybir
from concourse.tile import TileContext
from concourse.bass_utils import run_bass_kernel_spmd
from concourse import library_config

B, A, N, G, F = 32, 512, 64, 64, 128
NCORES = 8
FR = B // NCORES          # frames per core
ROWS = A * N              # rows (a, n) per frame = 32768
S = 8                     # slabs per frame
SLAB = ROWS // S          # 4096 rows per slab
NSW = 2                   # slabs per frame gathered via SWDGE dma_gather
SNO = S - NSW             # slabs per frame gathered via one-hot matmul

f32, bf16, i16 = mybir.dt.float32, mybir.dt.bfloat16, mybir.dt.int16


def _build_kernel():
    nc = bacc.Bacc("TRN2")
    nc.gpsimd.load_library(library_config.mlp)

    # host-prepped rbf: bf16, gaussians on partitions, row-pair packed:
    # xb[fr,s][g, c] = rbf_row(s*4096 + c)[g] ; xb[fr,s][64+g, c] = row(+2048)
    xb_in = nc.dram_tensor("xb", [FR, S, 128, SLAB // 2], bf16, kind="ExternalInput")
    featg_in = nc.dram_tensor("featg", [FR * A, F], bf16, kind="ExternalInput")
    featc_in = nc.dram_tensor("featc", [128, FR * 4 * F], bf16, kind="ExternalInput")
    oh_in = nc.dram_tensor("oh", [FR, SNO, 128, 4 * SLAB], bf16, kind="ExternalInput")
    gidx_in = nc.dram_tensor("gidx", [FR, NSW, 128, SLAB // 16], i16, kind="ExternalInput")
    w1_in = nc.dram_tensor("w1d", [128, F], bf16, kind="ExternalInput")
    w2_in = nc.dram_tensor("w2", [F, F], bf16, kind="ExternalInput")
    b1_in = nc.dram_tensor("b1", [F, 1], f32, kind="ExternalInput")
    b2_in = nc.dram_tensor("b2", [F, 1], f32, kind="ExternalInput")
    y_out = nc.dram_tensor("y", [FR, F, A], f32, kind="ExternalOutput")

    with TileContext(nc) as tc:
        with (
            tc.tile_pool(name="const", bufs=1) as constp,
            tc.tile_pool(name="sb", bufs=2) as sb,
            tc.tile_pool(name="wk", bufs=4) as wk,
            tc.tile_pool(name="psA", bufs=2, space="PSUM") as psA,
            tc.tile_pool(name="psB", bufs=2, space="PSUM") as psB,
        ):
            w1d = constp.tile([128, F], bf16)
            nc.sync.dma_start(out=w1d[:], in_=w1_in[:])
            w2 = constp.tile([F, F], bf16)
            nc.sync.dma_start(out=w2[:], in_=w2_in[:])
            b1c = constp.tile([F, 1], f32)
            nc.sync.dma_start(out=b1c[:], in_=b1_in[:])
            b2c = constp.tile([F, 1], f32)
            nc.sync.dma_start(out=b2c[:], in_=b2_in[:])
            featc = constp.tile([128, FR * 4 * F], bf16)
            nc.sync.dma_start(out=featc[:], in_=featc_in[:])

            for fr in range(FR):
                aggf = sb.tile([F, A], f32, tag="aggf")
                for s in range(S):
                    xbt = sb.tile([128, SLAB // 2], bf16, tag="xb")
                    nc.sync.dma_start(out=xbt[:], in_=xb_in[fr, s])

                    if s < NSW:
                        idxt = sb.tile([128, SLAB // 16], i16, tag="idxt")
                        nc.sync.dma_start(out=idxt[:], in_=gidx_in[fr, s])
                        gt = sb.tile([128, SLAB], bf16, tag="gt")
                        nc.gpsimd.dma_gather(
                            gt[:].rearrange("p (one n) -> p one n", one=1),
                            featg_in[:],
                            idxt[:],
                            SLAB,
                            SLAB,
                            F,
                            transpose=True,
                            single_packet=False,
                        )
                    else:
                        oht = sb.tile([128, 4 * SLAB], bf16, tag="oht")
                        nc.sync.dma_start(out=oht[:], in_=oh_in[fr, s - NSW])

                    prod = sb.tile([F, SLAB], bf16, tag="prod")
                    for bi in range(4):
                        # rows bi*1024 .. bi*1024+1023 of the slab
                        half = 0 if bi < 2 else 64
                        xcol = (bi % 2) * 1024
                        p1 = psA.tile([F, 1024], f32, tag="pg")
                        for q in range(2):
                            nc.tensor.matmul(
                                p1[:, 512 * q : 512 * (q + 1)],
                                lhsT=w1d[half : half + 64, :],
                                rhs=xbt[half : half + 64, xcol + 512 * q : xcol + 512 * (q + 1)],
                                start=True,
                                stop=True,
                                tile_position=(half, 0),
                            )
                        ht = wk.tile([F, 1024], bf16, tag="ht")
                        nc.scalar.activation(
                            out=ht[:],
                            in_=p1[:],
                            func=mybir.ActivationFunctionType.Tanh,
                            bias=b1c[:, 0:1],
                        )
                        p2 = psB.tile([F, 1024], f32, tag="p2")
                        for q in range(2):
                            nc.tensor.matmul(
                                p2[:, 512 * q : 512 * (q + 1)],
                                lhsT=w2[:],
                                rhs=ht[:, 512 * q : 512 * (q + 1)],
                                start=True,
                                stop=True,
                            )

                        if s < NSW:
                            gsrc = gt[:, 1024 * bi : 1024 * (bi + 1)]
                        else:
                            gps = psB.tile([F, 1024], f32, tag="p2")
                            for q in range(2):
                                for k in range(4):
                                    nc.tensor.matmul(
                                        gps[:, 512 * q : 512 * (q + 1)],
                                        lhsT=featc[:, (fr * 4 + k) * F : (fr * 4 + k + 1) * F],
                                        rhs=oht[:, k * SLAB + 1024 * bi + 512 * q : k * SLAB + 1024 * bi + 512 * (q + 1)],
                                        start=(k == 0),
                                        stop=(k == 3),
                                    )
                            gst = wk.tile([F, 1024], bf16, tag="gst")
                            nc.scalar.activation(
                                out=gst[:],
                                in_=gps[:],
                                func=mybir.ActivationFunctionType.Copy,
                            )
                            gsrc = gst[:]

                        nc.vector.scalar_tensor_tensor(
                            out=prod[:, 1024 * bi : 1024 * (bi + 1)],
                            in0=p2[:],
                            scalar=b2c[:, 0:1],
                            in1=gsrc,
                            op0=mybir.AluOpType.add,
                            op1=mybir.AluOpType.mult,
                        )

                    # 6-level pairwise tree: 64 neighbors -> 1 per atom
                    pv = prod[:]
                    width = 32
                    for lvl in range(6):
                        n2 = width  # surviving half-width at this level
                        src = pv.rearrange("p (a n) -> p a n", n=2 * n2)
                        if lvl < 5:
                            t = wk.tile([F, 64 * n2], bf16, tag=f"t{lvl}")
                            nc.vector.tensor_tensor(
                                out=t[:].rearrange("p (a n) -> p a n", n=n2),
                                in0=src[:, :, 0:n2],
                                in1=src[:, :, n2 : 2 * n2],
                                op=mybir.AluOpType.add,
                            )
                            pv = t[:]
                            width //= 2
                        else:
                            nc.vector.tensor_tensor(
                                out=aggf[:, 64 * s : 64 * (s + 1)].rearrange(
                                    "p (a n) -> p a n", n=1
                                ),
                                in0=src[:, :, 0:1],
                                in1=src[:, :, 1:2],
                                op=mybir.AluOpType.add,
                            )

                nc.sync.dma_start(out=y_out[fr], in_=aggf[:])

    nc.compile()
    return nc


_NC_CACHE = None


def _get_nc():
    global _NC_CACHE
    if _NC_CACHE is None:
        _NC_CACHE = _build_kernel()
    return _NC_CACHE


def _make_in_maps(features, rbf_expansion, neighbor_list, W1, b1, W2, b2):
    bf = ml_dtypes.bfloat16
    w1d = np.ascontiguousarray(np.concatenate([W1, W1], axis=0).astype(bf))
    w2 = np.ascontiguousarray(W2.astype(bf))
    b1c = np.ascontiguousarray(b1.astype(np.float32).reshape(F, 1))
    b2c = np.ascontiguousarray(b2.astype(np.float32).reshape(F, 1))

    rbf_bf = rbf_expansion.astype(bf)          # [B, A, N, G]
    feat_bf = features.astype(bf)              # [B, A, F]

    in_maps = []
    for core in range(NCORES):
        fsl = slice(core * FR, (core + 1) * FR)
        # xb layout: [FR, S, 128, 2048]
        r = rbf_bf[fsl].reshape(FR, S, 2, SLAB // 2, G)
        xb = np.ascontiguousarray(r.transpose(0, 1, 2, 4, 3)).reshape(
            FR, S, 128, SLAB // 2
        )
        featg = np.ascontiguousarray(feat_bf[fsl].reshape(FR * A, F))
        # featc[p, (fr,k,f)] = features[fr, 128k+p, f]
        featc = np.ascontiguousarray(
            feat_bf[fsl].reshape(FR, 4, 128, F).transpose(2, 0, 1, 3)
        ).reshape(128, FR * 4 * F)

        nl = np.asarray(neighbor_list[fsl]).astype(np.int64)  # [FR, A, N]
        nlf = nl.reshape(FR, ROWS)

        gidx = np.empty((FR, NSW, 128, SLAB // 16), dtype=np.int16)
        for fr in range(FR):
            for s in range(NSW):
                vals = (nlf[fr, s * SLAB : (s + 1) * SLAB] + fr * A).astype(np.int16)
                gidx[fr, s] = np.tile(vals.reshape(SLAB // 16, 16).T, (8, 1))

        oh = np.zeros((FR, SNO, 128, 4, SLAB), dtype=bf)
        cidx = np.arange(SLAB)
        for fr in range(FR):
            for si in range(SNO):
                j = nlf[fr, (si + NSW) * SLAB : (si + NSW + 1) * SLAB]
                oh[fr, si, j & 127, j >> 7, cidx] = 1.0
        oh = oh.reshape(FR, SNO, 128, 4 * SLAB)

        in_maps.append(
            {
                "xb": xb,
                "featg": featg,
                "featc": featc,
                "oh": oh,
                "gidx": gidx,
                "w1d": w1d,
                "w2": w2,
                "b1": b1c,
                "b2": b2c,
            }
        )
    return in_maps


def _run(in_maps, trace=False):
    nc = _get_nc()
    return run_bass_kernel_spmd(nc, in_maps, list(range(NCORES)), trace=trace)


def kernel(features, rbf_expansion, neighbor_list, W1, b1, W2, b2):
    in_maps = _make_in_maps(
        np.asarray(features), np.asarray(rbf_expansion), np.asarray(neighbor_list),
        np.asarray(W1), np.asarray(b1), np.asarray(W2), np.asarray(b2),
    )
    res = _run(in_maps).results
    out = np.empty((B, A, F), dtype=np.float32)
    for core in range(NCORES):
        y = np.asarray(res[core]["y"])  # [FR, F, A]
        out[core * FR : (core + 1) * FR] = y.transpose(0, 2, 1)
    return out


def _install_ntff_hook():
    """Provide antenv.axon_hooks + register the ctypes NTFF hook."""
    import types

    if "antenv.axon_hooks" not in sys.modules:
        mod = types.ModuleType("antenv.axon_hooks")
        store = {}
        mod.set_axon_ntff_profile_hook = lambda h: store.__setitem__("h", h)
        mod.get_axon_ntff_profile_hook = lambda: store.get("h")
        sys.modules["antenv.axon_hooks"] = mod
        import antenv

        antenv.axon_hooks = mod
    from antenv.axon_hooks import get_axon_ntff_profile_hook, set_axon_ntff_profile_hook

    if get_axon_ntff_profile_hook() is None:
        sys.path.insert(0, "/root/.axon_site")
        from trn_agent_boot.trn_boot import _ntff_profile_via_ctypes

        set_axon_ntff_profile_hook(
            _ntff_profile_via_ctypes("/opt/axon/libaxon_pjrt.so")
        )
    import concourse.bass_utils as bu

    bu.upload_artifacts = lambda tmpdir: f"file://{tmpdir}"


def kernel_traced(features, rbf_expansion, neighbor_list, W1, b1, W2, b2):
    """Like kernel() but also returns the profiled HW execution time (ns)."""
    _install_ntff_hook()
    in_maps = _make_in_maps(
        np.asarray(features), np.asarray(rbf_expansion), np.asarray(neighbor_list),
        np.asarray(W1), np.asarray(b1), np.asarray(W2), np.asarray(b2),
    )
    r = _run(in_maps, trace=True)
    out = np.empty((B, A, F), dtype=np.float32)
    for core in range(NCORES):
        y = np.asarray(r.results[core]["y"])
        out[core * FR : (core + 1) * FR] = y.transpose(0, 2, 1)
    return out, r.exec_time_ns


# revision 19
# speedup vs baseline: 1.1411x; 1.1411x over previous
"""Trainium2 Bass kernel for ContinuousFilterConv (SchNet cfconv-style).

Computes, for each frame b and atom a:
    filt  = tanh(rbf[b,a,:,:] @ W1 + b1) @ W2 + b2          # [N, F]
    out[b,a,:] = sum_n filt[n,:] * features[b, nl[b,a,n], :]

Sharding: data-parallel over the 32 frames -> 8 NeuronCores x 4 frames.

Per-core pipeline:
  - rbf is pre-cast to bf16 and pre-transposed on the host into the
    matmul operand layout (gaussian dim on partitions, row-pair packed),
    so it streams in via plain HWDGE DMA with no on-device transpose.
  - mm1 (K=64, two PE-quadrant matmuls) -> tanh+b1 on the scalar engine
    (bf16 out) -> mm2 vs W2 in bf16.
  - neighbor gather is split: NSW slabs/frame use the SWDGE dma_gather
    (gpsimd desc-gen bound, ~33us/slab); the rest use a one-hot matmul
    on the PE (host uploads a one-hot encoding of neighbor_list; the PE
    computes feat^T @ onehot in 4 K=128 passes), staged PSUM->SBUF bf16
    by the scalar engine.
  - one fused DVE op computes (mm2_psum + b2) * gathered, then a 6-level
    pairwise tensor_tensor tree (2x bf16 mode) reduces 64 neighbors per
    atom (tensor_reduce is 1x-only on DVE, the tree is faster).
  - output is written untransposed [F, A]; the host transposes.
"""
import sys

for _p in ("/opt/trn_rl_repo", "/root/.axon_site/_ro/trn_rl_repo"):
    if _p not in sys.path:
        sys.path.insert(0, _p)

import numpy as np
import ml_dtypes

import concourse.bacc as bacc
import concourse.mybir as mybir
from concourse.tile import TileContext
from concourse.bass_utils import run_bass_kernel_spmd
from concourse import library_config

B, A, N, G, F = 32, 512, 64, 64, 128
NCORES = 8
FR = B // NCORES          # frames per core
ROWS = A * N              # rows (a, n) per frame = 32768
S = 8                     # slabs per frame
SLAB = ROWS // S          # 4096 rows per slab
NSW = 2                   # slabs per frame gathered via SWDGE dma_gather
SNO = S - NSW             # slabs per frame gathered via one-hot matmul

f32, bf16, i16 = mybir.dt.float32, mybir.dt.bfloat16, mybir.dt.int16


def _build_kernel():
    nc = bacc.Bacc("TRN2")
    nc.gpsimd.load_library(library_config.mlp)

    # host-prepped rbf: bf16, gaussians on partitions, row-pair packed:
    # xb[fr,s][g, c] = rbf_row(s*4096 + c)[g] ; xb[fr,s][64+g, c] = row(+2048)
    xb_in = nc.dram_tensor("xb", [FR, S, 128, SLAB // 2], bf16, kind="ExternalInput")
    featg_in = nc.dram_tensor("featg", [FR * A, F], bf16, kind="ExternalInput")
    featc_in = nc.dram_tensor("featc", [128, FR * 4 * F], bf16, kind="ExternalInput")
    oh_in = nc.dram_tensor("oh", [FR, SNO, 128, 4 * SLAB], bf16, kind="ExternalInput")
    gidx_in = nc.dram_tensor("gidx", [FR, NSW, 128, SLAB // 16], i16, kind="ExternalInput")
    w1_in = nc.dram_tensor("w1d", [128, F], bf16, kind="ExternalInput")
    w2_in = nc.dram_tensor("w2", [F, F], bf16, kind="ExternalInput")
    b1_in = nc.dram_tensor("b1", [F, 1], f32, kind="ExternalInput")
    b2_in = nc.dram_tensor("b2", [F, 1], f32, kind="ExternalInput")
    y_out = nc.dram_tensor("y", [FR, F, A], f32, kind="ExternalOutput")

    with TileContext(nc) as tc:
        with (
            tc.tile_pool(name="const", bufs=1) as constp,
            tc.tile_pool(name="sb", bufs=2) as sb,
            tc.tile_pool(name="wk", bufs=4) as wk,
            tc.tile_pool(name="psA", bufs=2, space="PSUM") as psA,
            tc.tile_pool(name="psB", bufs=2, space="PSUM") as psB,
        ):
            w1d = constp.tile([128, F], bf16)
            nc.sync.dma_start(out=w1d[:], in_=w1_in[:])
            w2 = constp.tile([F, F], bf16)
            nc.sync.dma_start(out=w2[:], in_=w2_in[:])
            b1c = constp.tile([F, 1], f32)
            nc.sync.dma_start(out=b1c[:], in_=b1_in[:])
            b2c = constp.tile([F, 1], f32)
            nc.sync.dma_start(out=b2c[:], in_=b2_in[:])
            featc = constp.tile([128, FR * 4 * F], bf16)
            nc.sync.dma_start(out=featc[:], in_=featc_in[:])

            for fr in range(FR):
                aggf = sb.tile([F, A], f32, tag="aggf")
                for s in range(S):
                    xbt = sb.tile([128, SLAB // 2], bf16, tag="xb")
                    nc.sync.dma_start(out=xbt[:], in_=xb_in[fr, s])

                    if s < NSW:
                        idxt = sb.tile([128, SLAB // 16], i16, tag="idxt")
                        nc.sync.dma_start(out=idxt[:], in_=gidx_in[fr, s])
                        gt = sb.tile([128, SLAB], bf16, tag="gt")
                        nc.gpsimd.dma_gather(
                            gt[:].rearrange("p (one n) -> p one n", one=1),
                            featg_in[:],
                            idxt[:],
                            SLAB,
                            SLAB,
                            F,
                            transpose=True,
                            single_packet=False,
                        )
                    else:
                        oht = sb.tile([128, 4 * SLAB], bf16, tag="oht")
                        nc.sync.dma_start(out=oht[:], in_=oh_in[fr, s - NSW])

                    prod = sb.tile([F, SLAB], bf16, tag="prod")
                    for bi in range(4):
                        # rows bi*1024 .. bi*1024+1023 of the slab
                        half = 0 if bi < 2 else 64
                        xcol = (bi % 2) * 1024
                        p1 = psA.tile([F, 1024], f32, tag="pg")
                        for q in range(2):
                            nc.tensor.matmul(
                                p1[:, 512 * q : 512 * (q + 1)],
                                lhsT=w1d[half : half + 64, :],
                                rhs=xbt[half : half + 64, xcol + 512 * q : xcol + 512 * (q + 1)],
                                start=True,
                                stop=True,
                                tile_position=(half, 0),
                            )
                        ht = wk.tile([F, 1024], bf16, tag="ht")
                        nc.scalar.activation(
                            out=ht[:],
                            in_=p1[:],
                            func=mybir.ActivationFunctionType.Tanh,
                            bias=b1c[:, 0:1],
                        )
                        p2 = psB.tile([F, 1024], f32, tag="p2")
                        for q in range(2):
                            nc.tensor.matmul(
                                p2[:, 512 * q : 512 * (q + 1)],
                                lhsT=w2[:],
                                rhs=ht[:, 512 * q : 512 * (q + 1)],
                                start=True,
                                stop=True,
                            )

                        if s < NSW:
                            gsrc = gt[:, 1024 * bi : 1024 * (bi + 1)]
                        else:
                            gps = psA.tile([F, 1024], f32, tag="pg")
                            for q in range(2):
                                for k in range(4):
                                    nc.tensor.matmul(
                                        gps[:, 512 * q : 512 * (q + 1)],
                                        lhsT=featc[:, (fr * 4 + k) * F : (fr * 4 + k + 1) * F],
                                        rhs=oht[:, k * SLAB + 1024 * bi + 512 * q : k * SLAB + 1024 * bi + 512 * (q + 1)],
                                        start=(k == 0),
                                        stop=(k == 3),
                                    )
                            gst = wk.tile([F, 1024], bf16, tag="gst")
                            nc.scalar.activation(
                                out=gst[:],
                                in_=gps[:],
                                func=mybir.ActivationFunctionType.Copy,
                            )
                            gsrc = gst[:]

                        nc.vector.scalar_tensor_tensor(
                            out=prod[:, 1024 * bi : 1024 * (bi + 1)],
                            in0=p2[:],
                            scalar=b2c[:, 0:1],
                            in1=gsrc,
                            op0=mybir.AluOpType.add,
                            op1=mybir.AluOpType.mult,
                        )

                    # 6-level pairwise tree: 64 neighbors -> 1 per atom
                    pv = prod[:]
                    width = 32
                    for lvl in range(6):
                        n2 = width  # surviving half-width at this level
                        src = pv.rearrange("p (a n) -> p a n", n=2 * n2)
                        if lvl < 5:
                            t = wk.tile([F, 64 * n2], bf16, tag=f"t{lvl}")
                            nc.vector.tensor_tensor(
                                out=t[:].rearrange("p (a n) -> p a n", n=n2),
                                in0=src[:, :, 0:n2],
                                in1=src[:, :, n2 : 2 * n2],
                                op=mybir.AluOpType.add,
                            )
                            pv = t[:]
                            width //= 2
                        else:
                            nc.vector.tensor_tensor(
                                out=aggf[:, 64 * s : 64 * (s + 1)].rearrange(
                                    "p (a n) -> p a n", n=1
                                ),
                                in0=src[:, :, 0:1],
                                in1=src[:, :, 1:2],
                                op=mybir.AluOpType.add,
                            )

                nc.sync.dma_start(out=y_out[fr], in_=aggf[:])

    nc.compile()
    return nc


_NC_CACHE = None


def _get_nc():
    global _NC_CACHE
    if _NC_CACHE is None:
        _NC_CACHE = _build_kernel()
    return _NC_CACHE


def _make_in_maps(features, rbf_expansion, neighbor_list, W1, b1, W2, b2):
    bf = ml_dtypes.bfloat16
    w1d = np.ascontiguousarray(np.concatenate([W1, W1], axis=0).astype(bf))
    w2 = np.ascontiguousarray(W2.astype(bf))
    b1c = np.ascontiguousarray(b1.astype(np.float32).reshape(F, 1))
    b2c = np.ascontiguousarray(b2.astype(np.float32).reshape(F, 1))

    rbf_bf = rbf_expansion.astype(bf)          # [B, A, N, G]
    feat_bf = features.astype(bf)              # [B, A, F]

    in_maps = []
    for core in range(NCORES):
        fsl = slice(core * FR, (core + 1) * FR)
        # xb layout: [FR, S, 128, 2048]
        r = rbf_bf[fsl].reshape(FR, S, 2, SLAB // 2, G)
        xb = np.ascontiguousarray(r.transpose(0, 1, 2, 4, 3)).reshape(
            FR, S, 128, SLAB // 2
        )
        featg = np.ascontiguousarray(feat_bf[fsl].reshape(FR * A, F))
        # featc[p, (fr,k,f)] = features[fr, 128k+p, f]
        featc = np.ascontiguousarray(
            feat_bf[fsl].reshape(FR, 4, 128, F).transpose(2, 0, 1, 3)
        ).reshape(128, FR * 4 * F)

        nl = np.asarray(neighbor_list[fsl]).astype(np.int64)  # [FR, A, N]
        nlf = nl.reshape(FR, ROWS)

        gidx = np.empty((FR, NSW, 128, SLAB // 16), dtype=np.int16)
        for fr in range(FR):
            for s in range(NSW):
                vals = (nlf[fr, s * SLAB : (s + 1) * SLAB] + fr * A).astype(np.int16)
                gidx[fr, s] = np.tile(vals.reshape(SLAB // 16, 16).T, (8, 1))

        oh = np.zeros((FR, SNO, 128, 4, SLAB), dtype=bf)
        cidx = np.arange(SLAB)
        for fr in range(FR):
            for si in range(SNO):
                j = nlf[fr, (si + NSW) * SLAB : (si + NSW + 1) * SLAB]
                oh[fr, si, j & 127, j >> 7, cidx] = 1.0
        oh = oh.reshape(FR, SNO, 128, 4 * SLAB)

        in_maps.append(
            {
                "xb": xb,
                "featg": featg,
                "featc": featc,
                "oh": oh,
                "gidx": gidx,
                "w1d": w1d,
                "w2": w2,
                "b1": b1c,
                "b2": b2c,
            }
        )
    return in_maps


def _run(in_maps, trace=False):
    nc = _get_nc()
    return run_bass_kernel_spmd(nc, in_maps, list(range(NCORES)), trace=trace)


def kernel(features, rbf_expansion, neighbor_list, W1, b1, W2, b2):
    in_maps = _make_in_maps(
        np.asarray(features), np.asarray(rbf_expansion), np.asarray(neighbor_list),
        np.asarray(W1), np.asarray(b1), np.asarray(W2), np.asarray(b2),
    )
    res = _run(in_maps).results
    out = np.empty((B, A, F), dtype=np.float32)
    for core in range(NCORES):
        y = np.asarray(res[core]["y"])  # [FR, F, A]
        out[core * FR : (core + 1) * FR] = y.transpose(0, 2, 1)
    return out


def _install_ntff_hook():
    """Provide antenv.axon_hooks + register the ctypes NTFF hook."""
    import types

    if "antenv.axon_hooks" not in sys.modules:
        mod = types.ModuleType("antenv.axon_hooks")
        store = {}
        mod.set_axon_ntff_profile_hook = lambda h: store.__setitem__("h", h)
        mod.get_axon_ntff_profile_hook = lambda: store.get("h")
        sys.modules["antenv.axon_hooks"] = mod
        import antenv

        antenv.axon_hooks = mod
    from antenv.axon_hooks import get_axon_ntff_profile_hook, set_axon_ntff_profile_hook

    if get_axon_ntff_profile_hook() is None:
        sys.path.insert(0, "/root/.axon_site")
        from trn_agent_boot.trn_boot import _ntff_profile_via_ctypes

        set_axon_ntff_profile_hook(
            _ntff_profile_via_ctypes("/opt/axon/libaxon_pjrt.so")
        )
    import concourse.bass_utils as bu

    bu.upload_artifacts = lambda tmpdir: f"file://{tmpdir}"


def kernel_traced(features, rbf_expansion, neighbor_list, W1, b1, W2, b2):
    """Like kernel() but also returns the profiled HW execution time (ns)."""
    _install_ntff_hook()
    in_maps = _make_in_maps(
        np.asarray(features), np.asarray(rbf_expansion), np.asarray(neighbor_list),
        np.asarray(W1), np.asarray(b1), np.asarray(W2), np.asarray(b2),
    )
    r = _run(in_maps, trace=True)
    out = np.empty((B, A, F), dtype=np.float32)
    for core in range(NCORES):
        y = np.asarray(r.results[core]["y"])
        out[core * FR : (core + 1) * FR] = y.transpose(0, 2, 1)
    return out, r.exec_time_ns


# revision 21
# speedup vs baseline: 1.2569x; 1.1015x over previous
"""Trainium2 Bass kernel for ContinuousFilterConv (SchNet cfconv-style).

Computes, for each frame b and atom a:
    filt  = tanh(rbf[b,a,:,:] @ W1 + b1) @ W2 + b2          # [N, F]
    out[b,a,:] = sum_n filt[n,:] * features[b, nl[b,a,n], :]

Sharding: data-parallel over the 32 frames -> 8 NeuronCores x 4 frames.

Per-core pipeline:
  - rbf is pre-cast to bf16 and pre-transposed on the host into the
    matmul operand layout (gaussian dim on partitions, row-pair packed),
    so it streams in via plain HWDGE DMA with no on-device transpose.
  - mm1 (K=64, two PE-quadrant matmuls) -> tanh+b1 on the scalar engine
    (bf16 out) -> mm2 vs W2 in bf16.
  - neighbor gather is split: NSW slabs/frame use the SWDGE dma_gather
    (gpsimd desc-gen bound, ~33us/slab); the rest use a one-hot matmul
    on the PE (host uploads a one-hot encoding of neighbor_list; the PE
    computes feat^T @ onehot in 4 K=128 passes), staged PSUM->SBUF bf16
    by the scalar engine.
  - one fused DVE op computes (mm2_psum + b2) * gathered, then a 6-level
    pairwise tensor_tensor tree (2x bf16 mode) reduces 64 neighbors per
    atom (tensor_reduce is 1x-only on DVE, the tree is faster).
  - output is written untransposed [F, A]; the host transposes.
"""
import sys

for _p in ("/opt/trn_rl_repo", "/root/.axon_site/_ro/trn_rl_repo"):
    if _p not in sys.path:
        sys.path.insert(0, _p)

import numpy as np
import ml_dtypes

import concourse.bacc as bacc
import concourse.mybir as mybir
from concourse.tile import TileContext
from concourse.bass_utils import run_bass_kernel_spmd
from concourse import library_config

B, A, N, G, F = 32, 512, 64, 64, 128
NCORES = 8
FR = B // NCORES          # frames per core
ROWS = A * N              # rows (a, n) per frame = 32768
S = 8                     # slabs per frame
SLAB = ROWS // S          # 4096 rows per slab
NSW = 2                   # slabs per frame gathered via SWDGE dma_gather
SNO = S - NSW             # slabs per frame gathered via one-hot matmul

f32, bf16, i16 = mybir.dt.float32, mybir.dt.bfloat16, mybir.dt.int16


def _build_kernel():
    nc = bacc.Bacc("TRN2")
    nc.gpsimd.load_library(library_config.mlp)

    # host-prepped rbf: bf16, gaussians on partitions, row-pair packed:
    # xb[fr,s][g, c] = rbf_row(s*4096 + c)[g] ; xb[fr,s][64+g, c] = row(+2048)
    xb_in = nc.dram_tensor("xb", [FR, S, 128, SLAB // 2], bf16, kind="ExternalInput")
    featg_in = nc.dram_tensor("featg", [FR * A, F], bf16, kind="ExternalInput")
    featc_in = nc.dram_tensor("featc", [128, FR * 4 * F], bf16, kind="ExternalInput")
    oh_in = nc.dram_tensor("oh", [FR, SNO, 128, 4 * SLAB], bf16, kind="ExternalInput")
    gidx_in = nc.dram_tensor("gidx", [FR, NSW, 128, SLAB // 16], i16, kind="ExternalInput")
    w1_in = nc.dram_tensor("w1d", [128, F], bf16, kind="ExternalInput")
    w2_in = nc.dram_tensor("w2", [F, F], bf16, kind="ExternalInput")
    b1_in = nc.dram_tensor("b1", [F, 1], f32, kind="ExternalInput")
    b2_in = nc.dram_tensor("b2", [F, 1], f32, kind="ExternalInput")
    y_out = nc.dram_tensor("y", [FR, F, A], f32, kind="ExternalOutput")

    with TileContext(nc) as tc:
        with (
            tc.tile_pool(name="const", bufs=1) as constp,
            tc.tile_pool(name="sb", bufs=2) as sb,
            tc.tile_pool(name="gp", bufs=3) as gp,
            tc.tile_pool(name="wk", bufs=4) as wk,
            tc.tile_pool(name="psA", bufs=2, space="PSUM") as psA,
            tc.tile_pool(name="psB", bufs=2, space="PSUM") as psB,
        ):
            w1d = constp.tile([128, F], bf16)
            nc.sync.dma_start(out=w1d[:], in_=w1_in[:])
            w2 = constp.tile([F, F], bf16)
            nc.sync.dma_start(out=w2[:], in_=w2_in[:])
            b1c = constp.tile([F, 1], f32)
            nc.sync.dma_start(out=b1c[:], in_=b1_in[:])
            b2c = constp.tile([F, 1], f32)
            nc.sync.dma_start(out=b2c[:], in_=b2_in[:])
            featc = constp.tile([128, FR * 4 * F], bf16)
            nc.sync.dma_start(out=featc[:], in_=featc_in[:])

            for fr in range(FR):
                aggf = sb.tile([F, A], f32, tag="aggf")
                for s in range(S):
                    xbt = sb.tile([128, SLAB // 2], bf16, tag="xb")
                    nc.sync.dma_start(out=xbt[:], in_=xb_in[fr, s])

                    if s < NSW:
                        idxt = gp.tile([128, SLAB // 16], i16, tag="idxt")
                        nc.sync.dma_start(out=idxt[:], in_=gidx_in[fr, s])
                        gt = gp.tile([128, SLAB], bf16, tag="gt")
                        nc.gpsimd.dma_gather(
                            gt[:].rearrange("p (one n) -> p one n", one=1),
                            featg_in[:],
                            idxt[:],
                            SLAB,
                            SLAB,
                            F,
                            transpose=True,
                            single_packet=False,
                        )
                    else:
                        oht = sb.tile([128, 4 * SLAB], bf16, tag="oht")
                        nc.sync.dma_start(out=oht[:], in_=oh_in[fr, s - NSW])

                    prod = sb.tile([F, SLAB], bf16, tag="prod")
                    for bi in range(4):
                        # rows bi*1024 .. bi*1024+1023 of the slab
                        half = 0 if bi < 2 else 64
                        xcol = (bi % 2) * 1024
                        p1 = psA.tile([F, 1024], f32, tag="pg")
                        for q in range(2):
                            nc.tensor.matmul(
                                p1[:, 512 * q : 512 * (q + 1)],
                                lhsT=w1d[half : half + 64, :],
                                rhs=xbt[half : half + 64, xcol + 512 * q : xcol + 512 * (q + 1)],
                                start=True,
                                stop=True,
                                tile_position=(half, 0),
                            )
                        ht = wk.tile([F, 1024], bf16, tag="ht")
                        nc.scalar.activation(
                            out=ht[:],
                            in_=p1[:],
                            func=mybir.ActivationFunctionType.Tanh,
                            bias=b1c[:, 0:1],
                        )
                        p2 = psB.tile([F, 1024], f32, tag="p2")
                        for q in range(2):
                            nc.tensor.matmul(
                                p2[:, 512 * q : 512 * (q + 1)],
                                lhsT=w2[:],
                                rhs=ht[:, 512 * q : 512 * (q + 1)],
                                start=True,
                                stop=True,
                            )

                        if s < NSW:
                            gsrc = gt[:, 1024 * bi : 1024 * (bi + 1)]
                        else:
                            gps = psA.tile([F, 1024], f32, tag="pg")
                            for q in range(2):
                                for k in range(4):
                                    nc.tensor.matmul(
                                        gps[:, 512 * q : 512 * (q + 1)],
                                        lhsT=featc[:, (fr * 4 + k) * F : (fr * 4 + k + 1) * F],
                                        rhs=oht[:, k * SLAB + 1024 * bi + 512 * q : k * SLAB + 1024 * bi + 512 * (q + 1)],
                                        start=(k == 0),
                                        stop=(k == 3),
                                    )
                            gst = wk.tile([F, 1024], bf16, tag="gst")
                            nc.scalar.activation(
                                out=gst[:],
                                in_=gps[:],
                                func=mybir.ActivationFunctionType.Copy,
                            )
                            gsrc = gst[:]

                        nc.vector.scalar_tensor_tensor(
                            out=prod[:, 1024 * bi : 1024 * (bi + 1)],
                            in0=p2[:],
                            scalar=b2c[:, 0:1],
                            in1=gsrc,
                            op0=mybir.AluOpType.add,
                            op1=mybir.AluOpType.mult,
                        )

                    # 6-level pairwise tree: 64 neighbors -> 1 per atom
                    pv = prod[:]
                    width = 32
                    for lvl in range(6):
                        n2 = width  # surviving half-width at this level
                        src = pv.rearrange("p (a n) -> p a n", n=2 * n2)
                        if lvl < 5:
                            t = wk.tile([F, 64 * n2], bf16, tag=f"t{lvl}")
                            nc.vector.tensor_tensor(
                                out=t[:].rearrange("p (a n) -> p a n", n=n2),
                                in0=src[:, :, 0:n2],
                                in1=src[:, :, n2 : 2 * n2],
                                op=mybir.AluOpType.add,
                            )
                            pv = t[:]
                            width //= 2
                        else:
                            nc.vector.tensor_tensor(
                                out=aggf[:, 64 * s : 64 * (s + 1)].rearrange(
                                    "p (a n) -> p a n", n=1
                                ),
                                in0=src[:, :, 0:1],
                                in1=src[:, :, 1:2],
                                op=mybir.AluOpType.add,
                            )

                nc.sync.dma_start(out=y_out[fr], in_=aggf[:])

    nc.compile()
    return nc


_NC_CACHE = None


def _get_nc():
    global _NC_CACHE
    if _NC_CACHE is None:
        _NC_CACHE = _build_kernel()
    return _NC_CACHE


def _make_in_maps(features, rbf_expansion, neighbor_list, W1, b1, W2, b2):
    bf = ml_dtypes.bfloat16
    w1d = np.ascontiguousarray(np.concatenate([W1, W1], axis=0).astype(bf))
    w2 = np.ascontiguousarray(W2.astype(bf))
    b1c = np.ascontiguousarray(b1.astype(np.float32).reshape(F, 1))
    b2c = np.ascontiguousarray(b2.astype(np.float32).reshape(F, 1))

    rbf_bf = rbf_expansion.astype(bf)          # [B, A, N, G]
    feat_bf = features.astype(bf)              # [B, A, F]

    in_maps = []
    for core in range(NCORES):
        fsl = slice(core * FR, (core + 1) * FR)
        # xb layout: [FR, S, 128, 2048]
        r = rbf_bf[fsl].reshape(FR, S, 2, SLAB // 2, G)
        xb = np.ascontiguousarray(r.transpose(0, 1, 2, 4, 3)).reshape(
            FR, S, 128, SLAB // 2
        )
        featg = np.ascontiguousarray(feat_bf[fsl].reshape(FR * A, F))
        # featc[p, (fr,k,f)] = features[fr, 128k+p, f]
        featc = np.ascontiguousarray(
            feat_bf[fsl].reshape(FR, 4, 128, F).transpose(2, 0, 1, 3)
        ).reshape(128, FR * 4 * F)

        nl = np.asarray(neighbor_list[fsl]).astype(np.int64)  # [FR, A, N]
        nlf = nl.reshape(FR, ROWS)

        gidx = np.empty((FR, NSW, 128, SLAB // 16), dtype=np.int16)
        for fr in range(FR):
            for s in range(NSW):
                vals = (nlf[fr, s * SLAB : (s + 1) * SLAB] + fr * A).astype(np.int16)
                gidx[fr, s] = np.tile(vals.reshape(SLAB // 16, 16).T, (8, 1))

        oh = np.zeros((FR, SNO, 128, 4, SLAB), dtype=bf)
        cidx = np.arange(SLAB)
        for fr in range(FR):
            for si in range(SNO):
                j = nlf[fr, (si + NSW) * SLAB : (si + NSW + 1) * SLAB]
                oh[fr, si, j & 127, j >> 7, cidx] = 1.0
        oh = oh.reshape(FR, SNO, 128, 4 * SLAB)

        in_maps.append(
            {
                "xb": xb,
                "featg": featg,
                "featc": featc,
                "oh": oh,
                "gidx": gidx,
                "w1d": w1d,
                "w2": w2,
                "b1": b1c,
                "b2": b2c,
            }
        )
    return in_maps


def _run(in_maps, trace=False):
    nc = _get_nc()
    return run_bass_kernel_spmd(nc, in_maps, list(range(NCORES)), trace=trace)


def kernel(features, rbf_expansion, neighbor_list, W1, b1, W2, b2):
    in_maps = _make_in_maps(
        np.asarray(features), np.asarray(rbf_expansion), np.asarray(neighbor_list),
        np.asarray(W1), np.asarray(b1), np.asarray(W2), np.asarray(b2),
    )
    res = _run(in_maps).results
    out = np.empty((B, A, F), dtype=np.float32)
    for core in range(NCORES):
        y = np.asarray(res[core]["y"])  # [FR, F, A]
        out[core * FR : (core + 1) * FR] = y.transpose(0, 2, 1)
    return out


def _install_ntff_hook():
    """Provide antenv.axon_hooks + register the ctypes NTFF hook."""
    import types

    if "antenv.axon_hooks" not in sys.modules:
        mod = types.ModuleType("antenv.axon_hooks")
        store = {}
        mod.set_axon_ntff_profile_hook = lambda h: store.__setitem__("h", h)
        mod.get_axon_ntff_profile_hook = lambda: store.get("h")
        sys.modules["antenv.axon_hooks"] = mod
        import antenv

        antenv.axon_hooks = mod
    from antenv.axon_hooks import get_axon_ntff_profile_hook, set_axon_ntff_profile_hook

    if get_axon_ntff_profile_hook() is None:
        sys.path.insert(0, "/root/.axon_site")
        from trn_agent_boot.trn_boot import _ntff_profile_via_ctypes

        set_axon_ntff_profile_hook(
            _ntff_profile_via_ctypes("/opt/axon/libaxon_pjrt.so")
        )
    import concourse.bass_utils as bu

    bu.upload_artifacts = lambda tmpdir: f"file://{tmpdir}"


def kernel_traced(features, rbf_expansion, neighbor_list, W1, b1, W2, b2):
    """Like kernel() but also returns the profiled HW execution time (ns)."""
    _install_ntff_hook()
    in_maps = _make_in_maps(
        np.asarray(features), np.asarray(rbf_expansion), np.asarray(neighbor_list),
        np.asarray(W1), np.asarray(b1), np.asarray(W2), np.asarray(b2),
    )
    r = _run(in_maps, trace=True)
    out = np.empty((B, A, F), dtype=np.float32)
    for core in range(NCORES):
        y = np.asarray(r.results[core]["y"])
        out[core * FR : (core + 1) * FR] = y.transpose(0, 2, 1)
    return out, r.exec_time_ns
